# revision 32
# baseline (speedup 1.0000x reference)
"""Trainium2 Bass kernel for PixelUnshuffle->MHA->PixelShuffle (nn_Attention).

Reference computation (per batch element, 8 batch elements data-parallel
across 8 NeuronCores):
  x [64, 256, 256] --PixelUnshuffle(8)--> tokens [N=1024, C=4096]
  qkv = tokens @ W_qkv            [1024, 768]
  4-head attention (d=64), softmax over tokens
  y = attn_out @ W_out + b_out    [1024, 4096]
  --PixelShuffle(8)--> [64, 256, 256]

Layout strategy: all pixel un/shuffle permutations are done on the HOST
(numpy), so every device-side DMA and every PSUM->SBUF evacuation copy is
fully contiguous. x arrives pre-permuted+pre-cast as bf16 slices in
(w, cg, r2, p, hh, ww) order; the output leaves the device as bf16 in
(ct, nq, p, r2, hq, ww) order and the host undoes the permutation, adds
b_out and casts to f32. Weights are host-side pre-permuted/pre-cast.

Token index   n = hh*32 + ww            (hh, ww in [0,32))
Channel index c = c0*64 + r1*8 + r2     (c0 in [0,64), r1, r2 in [0,8))
x[c0, hh*8+r1, ww*8+r2] = tokens[n, c]
SBUF contraction partition p = (c0 % 16)*8 + r1, outer chunks cg = c0//16.

Schedule: stage 1 (QKV projection) runs r2-outer over two 512-token
windows; during the second window the first quarter of the attention
(dots+exp for m,n in window 0) is interleaved so the scalar engine's
exp work starts early.  Attention is computed transposed (dotsT[m, n]
with the summed token m on partitions); no on-chip transposes anywhere:
  dotsT[m,n] = kT_h[:, m-chunk] (lhsT) x qT_h (rhs)  -> exp -> edotsT
  out_augT[i', n] = sum_m v_aug[m, i'] * edotsT[m, n]
Even heads use v_aug with a ones column at 64 (Z lands on psum row 64,
outputs on rows 0..63); odd heads use a v block shifted to columns
64..127 with the ones column at 32, so their outputs land on psum rows
64..127 and both normalize multiplies write outT in place.  1/Z is
computed on 64 lanes (SBUF->SBUF DMA redistributes [1,512] -> [64,8]
before the reciprocal) and broadcast across partitions with a 0-stride
DRAM read; the multiplies run on the otherwise idle gpsimd engine.
The first three output-projection units are emitted inside the
attention PSUM pool so the PE stays busy through the final normalize
chains; the rest of stage 3 follows nq-outer/ic-outer so nothing waits
on the last head pair longer than necessary.
"""

import sys

if "/opt/trn_rl_repo" not in sys.path:
    sys.path.insert(0, "/opt/trn_rl_repo")

import os

import ml_dtypes
import numpy as np

import concourse.bass as bass
from concourse import bacc, mybir, tile
from concourse.bass_utils import run_bass_kernel_spmd

F32 = mybir.dt.float32
BF16 = mybir.dt.bfloat16

SCALE = 0.125  # DIM_HEAD ** -0.5

_CACHE = {}


def _build():
    nc = bacc.Bacc("TRN2", target_bir_lowering=False, debug=False, num_devices=8)

    # x pre-permuted on host: [w(2)*cg(4)*r2(8), p(128), hh(16)*ww(32)]
    x_d = nc.dram_tensor("x", [64, 128, 512], BF16, kind="ExternalInput").ap()
    wq_d = nc.dram_tensor("W_qkv", [4096, 768], BF16, kind="ExternalInput").ap()
    wo_d = nc.dram_tensor("W_out", [256, 4096], BF16, kind="ExternalInput").ap()
    # out: [ct(4)*nq(4), p(128), rh(2)*r4(4)*hq(8)*ww(32)] bf16, host unshuffles
    out_d = nc.dram_tensor("out", [16, 128, 2048], BF16, kind="ExternalOutput").ap()

    zrc_d = nc.dram_tensor("zr_scratch", [4, 1024], mybir.dt.float32).ap()

    def dram_ap(base, off, pattern):
        return bass.AP(tensor=base.tensor, offset=base.offset + off, ap=pattern)

    with tile.TileContext(nc) as tc:
        _build_tiled(nc, tc, x_d, wq_d, wo_d, out_d, zrc_d, dram_ap)
    nc.compile()
    return nc


def _build_tiled(nc, tc, x_d, wq_d, wo_d, out_d, zrc_d, dram_ap):
    from contextlib import ExitStack

    with ExitStack() as ctx:
        pers = ctx.enter_context(tc.tile_pool(name="pers", bufs=1))
        s23 = ctx.enter_context(tc.tile_pool(name="s23", bufs=1))
        sed = ctx.enter_context(tc.tile_pool(name="sed", bufs=1))
        s2m = ctx.enter_context(tc.tile_pool(name="s2m", bufs=1))
        s3p = ctx.enter_context(tc.tile_pool(name="s3p", bufs=1))

        # ---- persistent tiles ----
        # qkT[d-part, ot, n] : ot 0,1 = q dims 0..128,128..256; ot 2,3 = k
        qkT = pers.tile([128, 4, 1024], BF16)
        # even heads (h=2hp): v in cols 0..63, ones col 64 (65..67 pad)
        v_sb = pers.tile([128, 8, 2, 68], BF16)
        # odd heads (h=2hp+1): ones col 32 (Z), zeros elsewhere below 64,
        # v in cols 64..127 (psum base-partition rule: Z row must be 32)
        v2_sb = pers.tile([128, 8, 2, 128], BF16)
        # outT[i-part, ic, n] : i = h*64+d ; ic = i//128
        outT = pers.tile([128, 2, 1024], BF16)
        # W_out tile; DMA issued behind stage-1 loads
        wo_sb = s23.tile([128, 2, 4096], BF16)  # [i-part, ic, c_perm]

        nc.vector.memset(v_sb[:, :, :, 64:68], 1.0)
        nc.vector.memset(v2_sb[:], 0.0)
        nc.vector.memset(v2_sb[:, :, :, 32:33], 1.0)

        # exp(dots) tiles: ed[hp][h2][m-part, mc, n]
        ed = [
            [
                sed.tile(
                    [128, 8, 1024], BF16, tag=f"ed{hp}{h2}", bufs=1,
                    name=f"ed_{hp}_{h2}",
                )
                for h2 in range(2)
            ]
            for hp in range(2)
        ]

        # part A of attention: (m, n) both in window 0; emitted inside
        # stage-1's second window so exp starts early on the scalar engine
        partA = [
            (hp, mc, h2) for hp in range(2) for mc in range(4) for h2 in range(2)
        ]

        def emit_dots(pool, tag, hp, mc, h2, nh):
            b = h2 * 64
            dt = pool.tile(
                [128, 512], F32, tag=tag, bufs=2, name=f"dt_{hp}_{mc}_{h2}_{nh}"
            )
            nc.tensor.matmul(
                dt[:],
                qkT[b : b + 64, 2 + hp, mc * 128 : (mc + 1) * 128],
                qkT[b : b + 64, hp, nh * 512 : (nh + 1) * 512],
                start=True,
                stop=True,
            )
            nc.scalar.activation(
                ed[hp][h2][:, mc, nh * 512 : (nh + 1) * 512],
                dt[:],
                mybir.ActivationFunctionType.Exp,
                scale=SCALE,
            )

        # =========================== stage 1 ===========================
        # QKV projection, r2-outer over two 512-token windows.
        with (
            tc.tile_pool(name="wq", bufs=1) as wqp,
            tc.tile_pool(name="xw", bufs=1) as xwp,
            tc.tile_pool(name="ps1", bufs=1, space="PSUM") as ps1,
        ):
            wq_sb = wqp.tile([128, 8, 4, 768], BF16)  # [c-part, r2, cg, o]

            # PE warmup: dummy matmuls so HAM ramps toward 2.4 GHz before
            # the first real matmul arrives (DMA-gated, ~4us away)
            warm = wqp.tile([128, 512], BF16)
            nc.vector.memset(warm[:], 0.0)
            warm_ps = ps1.tile([128, 512], F32, tag="qk0", bufs=1)
            for i in range(10):
                nc.tensor.matmul(
                    warm_ps[:], warm[:, 0:128], warm[:], start=True, stop=True
                )

            # --- DMA issue: wq (+wo) on sync in priority order; transient
            # x slices round-robin scalar/gpsimd in consumption order ---
            for r2 in range(8):
                nc.sync.dma_start(
                    out=wq_sb[:, r2, :, :],
                    in_=dram_ap(
                        wq_d, r2 * 512 * 768, [[768, 128], [98304, 4], [1, 768]]
                    ),
                )
            nc.sync.dma_start(
                out=wo_sb[:],
                in_=dram_ap(wo_d, 0, [[4096, 128], [524288, 2], [1, 4096]]),
            )
            # transient x slices in consumption order, issued ~12 ahead
            xorder = [
                (w, r2, cg)
                for w in range(2)
                for r2 in range(8)
                for cg in range(4)
            ]
            xs = {}

            def issue_x(idx):
                if idx >= len(xorder):
                    return
                w, r2, cg = xorder[idx]
                t = xwp.tile(
                    [128, 16, 32], BF16, tag="xs", bufs=13,
                    name=f"xs_{w}_{r2}_{cg}",
                )
                eng = (nc.scalar, nc.gpsimd)[idx % 2]
                eng.dma_start(
                    out=t[:],
                    in_=dram_ap(
                        x_d,
                        (w * 32 + cg * 8 + r2) * 65536,
                        [[512, 128], [1, 512]],
                    ),
                )
                xs[(w, r2, cg)] = t

            for idx in range(12):
                issue_x(idx)

            for w in range(2):
                # psum: 4 qk banks + 2 packed v banks + 2 dtA banks
                qks = [
                    ps1.tile([128, 512], F32, tag=f"qk{ot}", bufs=1, name=f"qk_{w}_{ot}")
                    for ot in range(4)
                ]
                vps = [
                    ps1.tile([128, 2, 256], F32, tag=f"vp{i}", bufs=1, name=f"vp_{w}_{i}")
                    for i in range(2)
                ]
                for r2 in range(8):
                    for cg in range(4):
                        xtb = xs[(w, r2, cg)]
                        issue_x(w * 32 + r2 * 4 + cg + 12)
                        first = r2 == 0 and cg == 0
                        last = r2 == 7 and cg == 3
                        for ot in range(4):
                            nc.tensor.matmul(
                                qks[ot][:],
                                wq_sb[:, r2, cg, ot * 128 : (ot + 1) * 128],
                                xtb[:],
                                start=first,
                                stop=last,
                            )
                        for s in range(4):
                            # two v groups share a psum bank: the even
                            # group's `start` clears the whole bank, the
                            # odd group's first matmul overwrites its
                            # cleared half (start=False)
                            nc.tensor.matmul(
                                vps[s // 2][:, s % 2, :],
                                xtb[:, 4 * s : 4 * s + 4, :],
                                wq_sb[:, r2, cg, 512:768],
                                start=first and s % 2 == 0,
                                stop=last,
                            )
                    if w == 1:
                        for hp, mc, h2 in partA[r2 * 2 : r2 * 2 + 2]:
                            emit_dots(ps1, "dtA", hp, mc, h2, 0)
                for ot in range(4):
                    dst = qkT[:, ot, w * 512 : (w + 1) * 512]
                    if ot % 2 == 0:
                        nc.scalar.copy(dst, qks[ot][:])
                    else:
                        nc.vector.tensor_copy(dst, qks[ot][:])
                for s in range(4):
                    src4 = vps[s // 2][:, s % 2, :].rearrange(
                        "p (a b d) -> p a b d", a=2, b=2
                    )
                    nc.vector.tensor_copy(
                        v_sb[:, 4 * w + s, :, 0:64], src4[:, :, 0, :]
                    )
                    nc.vector.tensor_copy(
                        v2_sb[:, 4 * w + s, :, 64:128], src4[:, :, 1, :]
                    )

        def emit_y_unit(psp, nq, ct, ybufs=4):
            y_big = psp.tile(
                [128, 4, 256], F32, tag="ybig", bufs=ybufs, name=f"yb_{ct}_{nq}_a"
            )
            y_big2 = psp.tile(
                [128, 4, 256], F32, tag="ybig", bufs=ybufs, name=f"yb_{ct}_{nq}_b"
            )
            y_t = s3p.tile([128, 2048], BF16, tag="yt", bufs=6)
            for rh in range(2):
                yb = y_big if rh == 0 else y_big2
                # ic outer: the ic=0 half-chains depend only on outT[:,0]
                for ic in range(2):
                    for r4 in range(4):
                        r2 = rh * 4 + r4
                        nc.tensor.matmul(
                            yb[:, r4, :],
                            wo_sb[
                                :,
                                ic,
                                r2 * 512 + ct * 128 : r2 * 512 + (ct + 1) * 128,
                            ],
                            outT[:, ic, nq * 256 : (nq + 1) * 256],
                            start=(ic == 0 and r4 % 2 == 0),
                            stop=(ic == 1 and r4 % 2 == 1),
                        )
                # contiguous psum->sbuf cast; vector and scalar in parallel
                src = yb[:].rearrange("p a b -> p (a b)")
                dst = y_t[:, rh * 1024 : (rh + 1) * 1024]
                if rh == 0:
                    nc.vector.tensor_copy(dst, src)
                else:
                    nc.scalar.copy(dst, src)
            base = (ct * 4 + nq) * 262144
            nc.gpsimd.dma_start(
                out=dram_ap(out_d, base, [[2048, 128], [1, 1024]]),
                in_=y_t[:, 0:1024],
            )
            nc.sync.dma_start(
                out=dram_ap(out_d, base + 1024, [[2048, 128], [1, 1024]]),
                in_=y_t[:, 1024:2048],
            )


        # ======================= stage 2: attention =======================
        # Phase X: all remaining dots+exp units (scalar-bound; deep dt2
        # buffering keeps the PE fed in bursts).  Phase Y: all 64 AV
        # matmuls as one continuous PE stream into 8 single-bank per-nh
        # accumulators; each nh half normalizes as soon as it stops, its
        # banks free for stage 3 which chains straight in.
        with tc.tile_pool(name="psX", bufs=1, space="PSUM") as psX:

            def emit_units4(hp, h2, units):
                # 4 dots into one dt4 tile, one wide exp (less scalar
                # instruction overhead; scalar is the X-phase bottleneck)
                dt4 = psX.tile(
                    [128, 4, 512], F32, tag="dt4", bufs=2,
                    name=f"dt4_{hp}_{h2}_{units[0][0]}_{units[0][1]}",
                )
                b = h2 * 64
                for j, (mc, nh) in enumerate(units):
                    nc.tensor.matmul(
                        dt4[:, j, :],
                        qkT[b : b + 64, 2 + hp, mc * 128 : (mc + 1) * 128],
                        qkT[b : b + 64, hp, nh * 512 : (nh + 1) * 512],
                        start=True,
                        stop=True,
                    )
                return dt4

            for hp in range(2):
                for h2 in range(2):
                    # fine quad: mc 0-3, n-window 1
                    dt4 = emit_units4(hp, h2, [(mc, 1) for mc in range(4)])
                    nc.scalar.activation(
                        ed[hp][h2][:, 0:4, 512:1024],
                        dt4[:, :, :],
                        mybir.ActivationFunctionType.Exp,
                        scale=SCALE,
                    )
                    # coarse pairs: mc 4-5 and 6-7, both n-windows
                    for mcp in (4, 6):
                        dt4 = emit_units4(
                            hp, h2,
                            [(mcp, 0), (mcp, 1), (mcp + 1, 0), (mcp + 1, 1)],
                        )
                        nc.scalar.activation(
                            ed[hp][h2][:, mcp : mcp + 2, :],
                            dt4[:, :, :].rearrange("p a b -> p (a b)"),
                            mybir.ActivationFunctionType.Exp,
                            scale=SCALE,
                        )

        with tc.tile_pool(name="psY", bufs=1, space="PSUM") as psY:
            oaug = {}

            def get_oaug(hp, h2, nh):
                if (hp, h2, nh) not in oaug:
                    oaug[(hp, h2, nh)] = psY.tile(
                        [128, 512], F32, tag="oa", bufs=4,
                        name=f"oa_{hp}_{h2}_{nh}",
                    )
                return oaug[(hp, h2, nh)]

            def emit_av(hp, mc, h2, nh):
                lhs = v_sb[:, mc, hp, :] if h2 == 0 else v2_sb[:, mc, hp, :]
                dst = get_oaug(hp, h2, nh)
                dst = dst[0:68, :] if h2 == 0 else dst[:, :]
                nc.tensor.matmul(
                    dst,
                    lhs,
                    ed[hp][h2][:, mc, nh * 512 : (nh + 1) * 512],
                    start=(mc == 0),
                    stop=(mc == 7),
                )

            def emit_norm(hp, h2, nh):
                # spill the stopped nh-half (frees its psum bank), 1/Z via
                # [1,512]->[64,8] sbuf redistribute + 64-lane reciprocal +
                # DRAM 0-stride partition broadcast; multiply on gpsimd
                h = 2 * hp + h2
                zp = 64 if h2 == 0 else 32
                oa = oaug[(hp, h2, nh)]
                osb = s2m.tile(
                    [128, 512], F32, tag="osb", bufs=4, name=f"osb_{hp}_{h2}_{nh}"
                )
                if hp == 1:
                    nc.scalar.copy(osb[:], oa[:])
                else:
                    nc.vector.tensor_copy(osb[:], oa[:])
                z16 = s2m.tile([64, 8], F32, tag="z16", bufs=4)
                nc.sync.dma_start(out=z16[:], in_=osb[zp : zp + 1, :])
                z16r = s2m.tile([64, 8], F32, tag="z16r", bufs=4)
                nc.vector.reciprocal(z16r[:], z16[:])
                nc.sync.dma_start(
                    out=zrc_d[h, nh * 512 : (nh + 1) * 512].rearrange(
                        "(a b) -> a b", a=64
                    ),
                    in_=z16r[:],
                )
                zbc = s2m.tile([128, 512], F32, tag="zbc", bufs=4)
                nc.sync.dma_start(
                    out=zbc[:],
                    in_=dram_ap(zrc_d, h * 1024 + nh * 512, [[0, 128], [1, 512]]),
                )
                sl = slice(0, 64) if h2 == 0 else slice(64, 128)
                nc.gpsimd.tensor_mul(
                    outT[sl, hp, nh * 512 : (nh + 1) * 512],
                    osb[sl, :],
                    zbc[sl, :],
                )

            for nh in range(2):
                for hp in range(2):
                    for mc in range(8):
                        for h2 in range(2):
                            emit_av(hp, mc, h2, nh)
                for hp in range(2):
                    for h2 in range(2):
                        emit_norm(hp, h2, nh)
            # first output-projection unit rides in this pool: its psum
            # fits the freed nh0 accumulator banks and its matmuls cover
            # the nh1 normalize chain latency
            emit_y_unit(psY, 0, 0, ybufs=2)
            emit_y_unit(psY, 0, 1, ybufs=2)
            emit_y_unit(psY, 0, 2, ybufs=2)

        # ---------------- stage 3: output projection ----------------
        with tc.tile_pool(name="ps3", bufs=1, space="PSUM") as psY2:
            for nq in range(4):
                for ct in range(4):
                    if nq == 0 and ct < 3:
                        continue  # emitted early inside the attention pool
                    emit_y_unit(psY2, nq, ct)


def _get_nc():
    if "nc" not in _CACHE:
        _CACHE["nc"] = _build()
    return _CACHE["nc"]


def _prep_weights(W_qkv, W_out):
    wq_perm = np.ascontiguousarray(
        W_qkv.reshape(64, 8, 8, 768).transpose(2, 0, 1, 3).reshape(4096, 768)
    ).astype(ml_dtypes.bfloat16)
    wo_perm = np.ascontiguousarray(
        W_out.reshape(256, 64, 8, 8).transpose(0, 3, 1, 2).reshape(256, 4096)
    ).astype(ml_dtypes.bfloat16)
    return wq_perm, wo_perm


def _prep_x(x):
    """[8, 64, 256, 256] f32 -> [8, 64(w*32+cg*8+r2), 128, 512] bf16.

    xhost[b, w, cg, r2, p=(c0i*8+r1), hh, ww] =
        x[b, cg*16+c0i, (w*16+hh)*8+r1, ww*8+r2]
    """
    xb = np.asarray(x).astype(ml_dtypes.bfloat16)
    xb = xb.reshape(8, 4, 16, 2, 16, 8, 32, 8)  # [B, cg, c0i, w, hh, r1, ww, r2]
    xb = xb.transpose(0, 3, 1, 7, 2, 5, 4, 6)  # [B, w, cg, r2, c0i, r1, hh, ww]
    return np.ascontiguousarray(xb).reshape(8, 64, 128, 512)


def _post_out(o, b_out):
    """[16, 128, 2048] bf16 -> [64, 256, 256] f32 (+ bias)."""
    a = np.asarray(o).reshape(4, 4, 16, 8, 8, 8, 32)  # [ct,nq,phi,plo,r2,hq,ww]
    a = a.transpose(0, 2, 1, 5, 3, 6, 4)  # [ct,phi,nq,hq,plo,ww,r2]
    y = np.ascontiguousarray(a).astype(np.float32).reshape(64, 256, 256)
    if b_out is not None:
        # c = c0*64 + r1*8 + r2
        y.reshape(64, 32, 8, 32, 8)[:] += b_out.reshape(64, 1, 8, 1, 8)
    return y


def kernel(x, W_qkv, W_out, b_out):
    nc = _get_nc()
    wq_perm, wo_perm = _prep_weights(
        np.asarray(W_qkv, dtype=np.float32), np.asarray(W_out, dtype=np.float32)
    )
    xh = _prep_x(x)
    b_np = np.asarray(b_out, dtype=np.float32)
    if not np.any(b_np):
        b_np = None

    in_maps = [
        {"x": xh[b], "W_qkv": wq_perm, "W_out": wo_perm}
        for b in range(8)
    ]
    trace = bool(int(os.environ.get("BENCH_TRACE", "0")))
    if trace:
        try:  # tracing needs the NTFF hook shim (see test.py); degrade if absent
            from antenv.axon_hooks import get_axon_ntff_profile_hook  # noqa: F401
        except ImportError:
            trace = False
    res = run_bass_kernel_spmd(nc, in_maps, core_ids=list(range(8)), trace=trace)
    if trace:
        _CACHE["last_result"] = res
    return np.stack([_post_out(res.results[b]["out"], b_np) for b in range(8)])
